# revision 1
# baseline (speedup 1.0000x reference)
"""GCGRU cell (order-2 graph diffusion GRU) Trainium2 Bass kernel.

Strategy: data-parallel over batch (B=16 -> 2 batches per core x 8 cores).
Per core, activations are kept node-major ([node-chunk partitions x (b,c)
columns], fp16) so the graph-diffusion matmuls (contract over the node dim)
run with adj^T tiles as the PE-stationary operand, streamed from HBM and
accumulated over n-chunks in PSUM. The node dim is zero-padded to 4096 so
every tile is a full 128 partitions / 128 columns (fast weight load). adj^T
is host-retiled partition-major so each slab DMA is one dense transfer with
multi-KB contiguous runs per partition.

The gates share one diffusion of z=[x;h]; since z1=A z already contains A x,
the candidate path only diffuses r*h (128 batch-channel columns), using r*h as
the PE-stationary operand and adj as the 512-wide moving operand, producing
batch-major outputs that feed the candidate conv directly. The final candidate
conv + tanh + u*h+(1-u)*c combine is fused into the last diffusion's PSUM
group loop so the kernel tail is one group deep. sigmoid/tanh on ScalarE.
All input casts/layout transforms are done on host in kernel().
"""

import numpy as np

import concourse.bass as bass
from concourse import bacc
import concourse.mybir as mybir
import concourse.tile as tile
from concourse.bass_utils import run_bass_kernel_spmd

# problem constants
B, D_IN, D_H, NN = 16, 32, 64, 4000
NCORES = 8
B_LOC = B // NCORES          # batches per core
C = D_IN + D_H               # 96 channels into each gate conv
BC = B_LOC * C               # node-major column count (b-major: [b0 c96 | b1 c96])
BH = B_LOC * D_H             # stacked batch-hidden rows (128)
NP = 4096                    # node dim padded to a multiple of 128

F16 = mybir.dt.float16
F32 = mybir.dt.float32
CHUNK = 128


def build_program(npad=NP, nn=NN, mg=4, jb=8, nsl=512):
    """Build the single-core Bass program (same program runs SPMD on 8 cores).

    npad: padded node count; mg: m-chunks per PSUM group; jb: n-chunk blocks
    merged per slab DMA; nsl: node slice width for conv/elementwise loops.
    """
    chunk = CHUNK
    nch = npad // chunk          # node chunks
    ngrp = nch // mg             # psum groups per diffusion stage
    nsli = npad // nsl           # conv node slices
    assert nch % mg == 0 and npad % nsl == 0 and nch % jb == 0
    assert nsl == mg * chunk     # fused consumer: conv slice == psum group band
    assert BH == chunk

    nc = bacc.Bacc("TRN2", target_bir_lowering=False, debug=False)

    # ---- DRAM I/O (all host-prepped layouts) ----
    # at_t[g, p, j, :] = adjT[j*128+p, g*mg*128:(g+1)*mg*128]  (partition-major:
    # per partition, all n-chunk blocks of a group band are contiguous)
    at_d = nc.dram_tensor("at", [ngrp, chunk, nch, mg * chunk], F16,
                          kind="ExternalInput").ap()
    zt_d = nc.dram_tensor("zt", [npad, BC], F16, kind="ExternalInput").ap()
    xh_d = nc.dram_tensor("xh", [B_LOC, C, npad], F16, kind="ExternalInput").ap()
    h_d = nc.dram_tensor("h", [B_LOC, D_H, npad], F16, kind="ExternalInput").ap()
    wf_d = nc.dram_tensor("wf", [3, C, D_H], F16, kind="ExternalInput").ap()
    wu_d = nc.dram_tensor("wu", [3, C, D_H], F16, kind="ExternalInput").ap()
    # candidate weights: x rows per diffusion order, and batch-duplicated rh rows
    wcx_d = nc.dram_tensor("wcx", [3, D_IN, D_H], F16, kind="ExternalInput").ap()
    wcrh_d = nc.dram_tensor("wcrh", [3, BH, D_H], F16, kind="ExternalInput").ap()
    bf_d = nc.dram_tensor("bf", [BH, 1], F32, kind="ExternalInput").ap()
    bu_d = nc.dram_tensor("bu", [BH, 1], F32, kind="ExternalInput").ap()
    bc_d = nc.dram_tensor("bcb", [BH, 1], F32, kind="ExternalInput").ap()
    id_d = nc.dram_tensor("idm", [chunk, chunk], F16, kind="ExternalInput").ap()
    out_d = nc.dram_tensor("out", [B_LOC, D_H, nn], F32, kind="ExternalOutput").ap()

    with tile.TileContext(nc) as tc:
        _body(tc, locals())
    nc.compile()
    return nc


def _body(tc, aps):
    nc = tc.nc
    npad, nn, chunk, mg, jb, nsl = (aps[k] for k in
                                    ("npad", "nn", "chunk", "mg", "jb", "nsl"))
    nch, ngrp, nsli = aps["nch"], aps["ngrp"], aps["nsli"]
    at_d, zt_d, xh_d, h_d = aps["at_d"], aps["zt_d"], aps["xh_d"], aps["h_d"]
    wf_d, wu_d, wcx_d, wcrh_d = (
        aps["wf_d"], aps["wu_d"], aps["wcx_d"], aps["wcrh_d"])
    bf_d, bu_d, bc_d, id_d, out_d = (
        aps["bf_d"], aps["bu_d"], aps["bc_d"], aps["id_d"], aps["out_d"])

    SIG = mybir.ActivationFunctionType.Sigmoid
    TANH = mybir.ActivationFunctionType.Tanh

    with (
        tc.tile_pool(name="const", bufs=1) as cpool,       # persistent small tiles
        tc.tile_pool(name="perst", bufs=1) as ppool,       # persistent activations
        tc.tile_pool(name="nmrot", bufs=2) as nmpool,      # rotating node-major tensors
        tc.tile_pool(name="cmrot", bufs=4) as cmpool,      # rotating channel-major tensors
        tc.tile_pool(name="slab", bufs=5) as slpool,       # adj slabs
        tc.tile_pool(name="psum", bufs=8, space="PSUM") as pspool,
        tc.tile_pool(name="stage", bufs=2) as stpool,      # small staging tiles
    ):
        # ---- persistent loads ----
        idm = cpool.tile([chunk, chunk], F16, tag="idm")
        nc.sync.dma_start(out=idm[:], in_=id_d[:])
        wf_sb = [cpool.tile([C, D_H], F16, tag=f"wf{k}", name=f"wf{k}")
                 for k in range(3)]
        wu_sb = [cpool.tile([C, D_H], F16, tag=f"wu{k}", name=f"wu{k}")
                 for k in range(3)]
        wcx_sb = [cpool.tile([D_IN, D_H], F16, tag=f"wcx{k}", name=f"wcx{k}")
                  for k in range(3)]
        wcrh_sb = [cpool.tile([BH, D_H], F16, tag=f"wcrh{k}", name=f"wcrh{k}")
                   for k in range(3)]
        for k in range(3):
            nc.scalar.dma_start(out=wf_sb[k][:], in_=wf_d[k])
            nc.scalar.dma_start(out=wu_sb[k][:], in_=wu_d[k])
            nc.scalar.dma_start(out=wcx_sb[k][:], in_=wcx_d[k])
            nc.scalar.dma_start(out=wcrh_sb[k][:], in_=wcrh_d[k])
        bf_sb = cpool.tile([BH, 1], F32, tag="bf")
        nc.sync.dma_start(out=bf_sb[:], in_=bf_d[:])
        bu_sb = cpool.tile([BH, 1], F32, tag="bu")
        nc.sync.dma_start(out=bu_sb[:], in_=bu_d[:])
        bc_sb = cpool.tile([BH, 1], F32, tag="bc")
        nc.sync.dma_start(out=bc_sb[:], in_=bc_d[:])

        # node-major [x;h]: one tile, chunk j occupies cols [j*BC, (j+1)*BC)
        # (rotating pool: ztT is dead after the first diffusion, z2T reuses it)
        ztT = nmpool.tile([chunk, nch * BC], F16, tag="nm", name="ztT")
        nc.sync.dma_start(
            out=ztT[:, :].rearrange("p (j f) -> p j f", j=nch),
            in_=zt_d[:, :].rearrange("(j p) f -> p j f", p=chunk))

        xh_sb = [ppool.tile([C, npad], F16, tag=f"xh{b}", name=f"xh{b}")
                 for b in range(B_LOC)]
        for b in range(B_LOC):
            nc.scalar.dma_start(out=xh_sb[b][:], in_=xh_d[b])
        # batch-stacked [b0 rows 0:64 | b1 rows 64:128]
        h_st = ppool.tile([BH, npad], F16, tag="h_st")
        for b in range(B_LOC):
            nc.scalar.dma_start(out=h_st[b * D_H:(b + 1) * D_H, :], in_=h_d[b])
        u_st = ppool.tile([BH, npad], F16, tag="u_st")
        rh_st = ppool.tile([BH, npad], F16, tag="rh_st")

        # ---- helpers ----
        def load_slab(g, jB):
            # two triggers per slab, one per HWDGE ring (SP + ACT), so both
            # trigger queues and transfer paths run in parallel
            slab = slpool.tile([chunk, jb * mg * chunk], F16, tag="slab",
                               name="slab")
            h1 = jb // 2
            eng2 = nc.scalar
            nc.sync.dma_start(
                out=slab[:, 0:h1 * mg * chunk].rearrange(
                    "p (j m) -> p j m", j=h1),
                in_=at_d[g, :, jB * jb: jB * jb + h1, :])
            eng2.dma_start(
                out=slab[:, h1 * mg * chunk:].rearrange(
                    "p (j m) -> p j m", j=jb - h1),
                in_=at_d[g, :, jB * jb + h1:(jB + 1) * jb, :])
            return slab

        def diffusion_sa(src, dst):
            """dst = A @ src, node-major -> node-major (adj stationary)."""
            for g in range(ngrp):
                pss = [pspool.tile([chunk, BC], F32, tag="ps", name=f"psd{mi}")
                       for mi in range(mg)]
                for jB in range(nch // jb):
                    slab = load_slab(g, jB)
                    for jj in range(jb):
                        j = jB * jb + jj
                        for mi in range(mg):
                            nc.tensor.matmul(
                                pss[mi][:, :],
                                lhsT=slab[:, (jj * mg + mi) * chunk:
                                          (jj * mg + mi + 1) * chunk],
                                rhs=src[:, j * BC:(j + 1) * BC],
                                start=(j == 0), stop=(j == nch - 1))
                for mi in range(mg):
                    m = g * mg + mi
                    nc.vector.tensor_copy(
                        out=dst[:, m * BC:(m + 1) * BC], in_=pss[mi][:, :])

        def diffusion_sz(src_nm, dst_bm, consumer=None):
            """dst_bm[128 bc, m] = (A @ src)^T with src (node-major [n, 128bc])
            stationary and adj moving. Optionally calls consumer(g) after the
            group band [g*nsl, (g+1)*nsl) of dst_bm is available."""
            for g in range(ngrp):
                psc = pspool.tile([BH, mg * chunk], F32, tag="ps", name="psz")
                for jB in range(nch // jb):
                    slab = load_slab(g, jB)
                    for jj in range(jb):
                        j = jB * jb + jj
                        nc.tensor.matmul(
                            psc[:, :],
                            lhsT=src_nm[:, j * chunk:(j + 1) * chunk],
                            rhs=slab[:, jj * mg * chunk:(jj + 1) * mg * chunk],
                            start=(j == 0), stop=(j == nch - 1))
                nc.vector.tensor_copy(
                    out=dst_bm[:, g * mg * chunk:(g + 1) * mg * chunk],
                    in_=psc[:, :])
                if consumer is not None:
                    consumer(g)

        def to_channel_major(src_nm):
            """node-major [chunk, nch*BC] fp16 -> per-batch channel-major [C, npad]."""
            cms = [cmpool.tile([C, npad], F16, tag="cm", name=f"cm{b}")
                   for b in range(B_LOC)]
            for b in range(B_LOC):
                for j in range(nch):
                    pt = pspool.tile([C, chunk], F16, tag="ps")
                    nc.tensor.transpose(
                        pt[:, :],
                        src_nm[:, j * BC + b * C: j * BC + (b + 1) * C],
                        idm[:, :])
                    nc.vector.tensor_copy(
                        out=cms[b][:, j * chunk:(j + 1) * chunk], in_=pt[:, :])
            return cms

        # ---- gates path: z1 = A z, z2 = A z1 ----
        z1T = nmpool.tile([chunk, nch * BC], F16, tag="nm")
        diffusion_sa(ztT, z1T)
        z2T = nmpool.tile([chunk, nch * BC], F16, tag="nm")
        diffusion_sa(z1T, z2T)

        z1cm = to_channel_major(z1T)
        z2cm = to_channel_major(z2T)

        # gate convs: r and u, batch-stacked in PSUM partitions
        # (rhT: node-major r*h, filled per band inside the loop)
        rhT = ppool.tile([chunk, nch * BH], F16, tag="rhT")
        for s in range(nsli):
            sl = slice(s * nsl, (s + 1) * nsl)
            psf = pspool.tile([BH, nsl], F32, tag="ps", name="psf")
            psu = pspool.tile([BH, nsl], F32, tag="ps", name="psu")
            for b in range(B_LOC):
                rows = slice(b * D_H, (b + 1) * D_H)
                feats = (xh_sb[b][:, sl], z1cm[b][:, sl], z2cm[b][:, sl])
                for k in range(3):
                    nc.tensor.matmul(psf[rows, :], lhsT=wf_sb[k], rhs=feats[k],
                                     start=(k == 0), stop=(k == 2))
                for k in range(3):
                    nc.tensor.matmul(psu[rows, :], lhsT=wu_sb[k], rhs=feats[k],
                                     start=(k == 0), stop=(k == 2))
            rst = stpool.tile([BH, nsl], F16, tag="rst")
            nc.scalar.activation(rst[:, :], psf[:, :], SIG, bias=bf_sb[:, :])
            nc.vector.tensor_mul(out=rh_st[:, sl], in0=rst[:, :],
                                 in1=h_st[:, sl])
            nc.scalar.activation(u_st[:, sl], psu[:, :], SIG, bias=bu_sb[:, :])
            # rhT transposes for this node band, so the candidate diffusion
            # can start as soon as the band is ready
            for b in range(B_LOC):
                rows = slice(b * D_H, (b + 1) * D_H)
                for j in range(s * nsl // chunk, (s + 1) * nsl // chunk):
                    pt = pspool.tile([chunk, D_H], F16, tag="ps", name="ptr")
                    nc.tensor.transpose(
                        pt[:, :], rh_st[rows, j * chunk:(j + 1) * chunk],
                        idm[rows, rows])
                    nc.vector.tensor_copy(
                        out=rhT[:, j * BH + b * D_H: j * BH + (b + 1) * D_H],
                        in_=pt[:, :])

        zc1_bm = ppool.tile([BH, npad], F16, tag="zc1bm")
        diffusion_sz(rhT, zc1_bm)

        zc1T = ppool.tile([chunk, nch * BH], F16, tag="zc1T")
        for j in range(nch):
            pt = pspool.tile([chunk, chunk], F16, tag="ps")
            nc.tensor.transpose(pt[:, :],
                                zc1_bm[:, j * chunk:(j + 1) * chunk], idm[:, :])
            nc.vector.tensor_copy(
                out=zc1T[:, j * chunk:(j + 1) * chunk], in_=pt[:, :])

        zc2_bm = ppool.tile([BH, npad], F16, tag="zc2bm")

        def consumer(s):
            # candidate conv for node band s, then out = c + u*(h-c)
            sl = slice(s * nsl, (s + 1) * nsl)
            psc2 = pspool.tile([BH, nsl], F32, tag="ps", name="psc2")
            for b in range(B_LOC):
                rows = slice(b * D_H, (b + 1) * D_H)
                terms = ((wcx_sb[0], xh_sb[b][0:D_IN, sl]),
                         (wcx_sb[1], z1cm[b][0:D_IN, sl]),
                         (wcx_sb[2], z2cm[b][0:D_IN, sl]),
                         (wcrh_sb[0][rows, :], rh_st[rows, sl]),
                         (wcrh_sb[1][rows, :], zc1_bm[rows, sl]),
                         (wcrh_sb[2][rows, :], zc2_bm[rows, sl]))
                for k, (wt, rhs) in enumerate(terms):
                    nc.tensor.matmul(psc2[rows, :], lhsT=wt, rhs=rhs,
                                     start=(k == 0), stop=(k == len(terms) - 1))
            cst = stpool.tile([BH, nsl], F32, tag="cst")
            nc.scalar.activation(cst[:, :], psc2[:, :], TANH, bias=bc_sb[:, :])
            t1 = stpool.tile([BH, nsl], F32, tag="t1")
            nc.vector.tensor_sub(out=t1[:, :], in0=h_st[:, sl], in1=cst[:, :])
            nc.vector.tensor_mul(out=t1[:, :], in0=u_st[:, sl], in1=t1[:, :])
            ost = stpool.tile([BH, nsl], F32, tag="ost")
            nc.vector.tensor_add(out=ost[:, :], in0=cst[:, :], in1=t1[:, :])
            w = min(nsl, nn - s * nsl)
            if w > 0:
                for b in range(B_LOC):
                    nc.scalar.dma_start(
                        out=out_d[b][:, s * nsl: s * nsl + w],
                        in_=ost[b * D_H:(b + 1) * D_H, 0:w])

        diffusion_sz(zc1T, zc2_bm, consumer=consumer)


# ---- host-side driver ----
_CACHED_NC = None
TRACE = False           # set True (e.g. from test.py) to capture an NTFF profile
TRACE_DIR = None
LAST_RESULTS = None     # BassKernelResults of the most recent kernel() call


def _host_prep(x, h, adj, Wf, bf, Wu, bu, Wc, bc, npad=NP, nn=NN, mg=4):
    """Shard + cast + layout inputs for the 8 cores. Returns list of in_maps."""
    chunk = CHUNK
    nch = npad // chunk
    ngrp = nch // mg
    # adj^T zero-padded to [npad, npad], retiled partition-major per group band
    at = np.zeros((npad, npad), dtype=np.float16)
    at[:nn, :nn] = adj.T.astype(np.float16)
    at_t = np.ascontiguousarray(
        at.reshape(nch, chunk, ngrp, mg * chunk).transpose(2, 1, 0, 3))
    idm = np.eye(chunk, dtype=np.float16)

    def wsplit(W):
        WT = W.T.astype(np.float16)                            # [3C, D_H]
        return np.ascontiguousarray(WT.reshape(3, C, D_H))

    wf3, wu3, wc3 = wsplit(Wf), wsplit(Wu), wsplit(Wc)
    wcx3 = np.ascontiguousarray(wc3[:, :D_IN])                 # [3, D_IN, D_H]
    wcrh = wc3[:, D_IN:]                                       # [3, D_H, D_H]
    wcrh3 = np.ascontiguousarray(
        np.concatenate([wcrh] * B_LOC, axis=1))                # [3, BH, D_H]

    def bstack(v):
        return np.concatenate([v] * B_LOC).reshape(BH, 1).astype(np.float32)

    shared = {
        "wf": wf3, "wu": wu3, "wcx": wcx3, "wcrh": wcrh3,
        "bf": bstack(bf), "bu": bstack(bu), "bcb": bstack(bc),
        "idm": idm, "at": at_t,
    }
    xh = np.concatenate([x, h], axis=1).astype(np.float16)     # [B, C, nn]
    xh_p = np.zeros((B, C, npad), dtype=np.float16)
    xh_p[:, :, :nn] = xh
    h_p = np.zeros((B, D_H, npad), dtype=np.float16)
    h_p[:, :, :nn] = h.astype(np.float16)
    in_maps = []
    for core in range(NCORES):
        bs = slice(core * B_LOC, (core + 1) * B_LOC)
        xh_c = xh_p[bs]                                        # [B_LOC, C, npad]
        zt_c = np.ascontiguousarray(
            xh_c.transpose(2, 0, 1).reshape(npad, B_LOC * C))
        in_maps.append(dict(shared, zt=zt_c,
                            xh=np.ascontiguousarray(xh_c),
                            h=np.ascontiguousarray(h_p[bs])))
    return in_maps


def kernel(**inputs):
    global _CACHED_NC, LAST_RESULTS
    inputs = {k: np.asarray(v) for k, v in inputs.items()}
    if _CACHED_NC is None:
        _CACHED_NC = build_program()
    in_maps = _host_prep(**inputs)
    kw = {}
    if TRACE:
        kw = dict(trace=True, tmpdir=TRACE_DIR)
    res = run_bass_kernel_spmd(_CACHED_NC, in_maps,
                               core_ids=list(range(NCORES)), **kw)
    LAST_RESULTS = res
    outs = [res.results[i]["out"] for i in range(NCORES)]
    return np.concatenate(outs, axis=0).astype(np.float32)


if __name__ == "__main__":
    rng = np.random.default_rng(0)
    ins = {
        "x": rng.standard_normal((B, D_IN, NN), dtype=np.float32),
        "h": rng.standard_normal((B, D_H, NN), dtype=np.float32),
        "adj": rng.random((NN, NN), dtype=np.float32) / NN,
        "Wf": rng.standard_normal((D_H, 3 * C), dtype=np.float32) * 0.05,
        "Wu": rng.standard_normal((D_H, 3 * C), dtype=np.float32) * 0.05,
        "Wc": rng.standard_normal((D_H, 3 * C), dtype=np.float32) * 0.05,
        "bf": rng.standard_normal(D_H).astype(np.float32) * 0.05,
        "bu": rng.standard_normal(D_H).astype(np.float32) * 0.05,
        "bc": rng.standard_normal(D_H).astype(np.float32) * 0.05,
    }
    out = kernel(**ins)
    print(out.shape, out.dtype)



# revision 10
# speedup vs baseline: 1.6692x; 1.6692x over previous
"""GCGRU cell (order-2 graph diffusion GRU) Trainium2 Bass kernel, v2.

Strategy: data-parallel over batch (B=16 -> 2 batches per core x 8 cores).
The whole adjacency matrix lives RESIDENT in SBUF as scaled fp8 (16.8 MB =
128 KiB/partition), loaded once instead of being streamed 4x in fp16. All
four diffusion stages (z1 = A z, z2 = A z1, zc1 = A rh, zc2 = A zc1) run as
fp8 DoubleRow matmuls (K=256 per pass, FD=512) in sz-form: the activation
tensor is PE-stationary (node-major, fp8, DR-interleaved) and adj is the
wide moving operand. Outputs are born channel-major, feeding the 1x1 gate
convs (fp16) directly with no PE transposes. Node-major fp8 stationaries
for later stages are produced by DMA XBAR transposes + DVE casts, so the
transpose path costs no PE time. Candidate x-part conv terms are
pre-accumulated into c_x during the gate band loop; the final tanh/combine
is fused into stage D's group loop.

Scales (static): adj x2^14 (below fp8 subnormal range otherwise), z x8,
z1 x128 / rh x16 / zc1 x128 at their fp8 casts; each undone in the psum
descale of the following stage. Diffused features are ~100x smaller than
the direct z term in the convs, so fp8 error there is negligible.
"""

import numpy as np
import ml_dtypes

import concourse.bass as bass
from concourse import bacc
import concourse.mybir as mybir
import concourse.tile as tile
from concourse.bass_utils import run_bass_kernel_spmd

# problem constants
B, D_IN, D_H, NN = 16, 32, 64, 4000
NCORES = 8
B_LOC = B // NCORES          # batches per core
C = D_IN + D_H               # 96 channels into each gate conv
BH = B_LOC * D_H             # stacked batch-hidden rows (128)
NP = 4096                    # node dim padded to a multiple of 512
CH = 128                     # node chunk
NCHUNK = NP // CH            # 32 node chunks
NJ2 = NCHUNK // 2            # 16 DoubleRow chunk pairs
BAND = 512                   # psum group band (m-range per diffusion group)
NG = NP // BAND              # 8 groups
NP8 = ml_dtypes.float8_e4m3  # TRN fp8_e4m3

F8 = mybir.dt.float8e4
F16 = mybir.dt.float16
F32 = mybir.dt.float32
DR = mybir.MatmulPerfMode.DoubleRow

# static scales
S_A = float(2 ** 14)
S_Z = 8.0
S_Z1 = 128.0
S_RH = 16.0
S_ZC1 = 128.0
CA = 1.0 / (S_A * S_Z)       # stage A psum descale
CB = 1.0 / (S_A * S_Z1)      # stage B
CC = 1.0 / (S_A * S_RH)      # stage C
CD = 1.0 / (S_A * S_ZC1)     # stage D


def build_program():
    nc = bacc.Bacc("TRN2", target_bir_lowering=False, debug=False)

    # ---- DRAM I/O (all host-prepped layouts) ----
    # at8[p, ((g*NJ2+j2)*2+i)*BAND + m] = S_A * adj[g*BAND+m, j2*256+i*128+p]
    at_d = nc.dram_tensor("at", [CH, NG * NJ2 * 2 * BAND], F8,
                          kind="ExternalInput").ap()
    # ztdr[p, j*192 + b*96 + c] = S_Z * z[b, c, j*128+p],  z = [x;h]
    zt_d = nc.dram_tensor("zt", [CH, NCHUNK * B_LOC * C], F8,
                          kind="ExternalInput").ap()
    xh_d = nc.dram_tensor("xh", [B_LOC, C, NP], F16, kind="ExternalInput").ap()
    hbs_d = nc.dram_tensor("hbs", [BH, NP], F16, kind="ExternalInput").ap()
    # gate weights: [f,u] x diffusion order; rows = contraction channels
    wg0_d = nc.dram_tensor("wg0", [2, C, D_H], F16, kind="ExternalInput").ap()
    wg1_d = nc.dram_tensor("wg1", [2, C, D_H], F16, kind="ExternalInput").ap()
    wg2_d = nc.dram_tensor("wg2", [2, C, D_H], F16, kind="ExternalInput").ap()
    # candidate x-part weights (orders 0,1,2; rows = x channels)
    wcx_d = nc.dram_tensor("wcx", [3, D_IN, D_H], F16, kind="ExternalInput").ap()
    # candidate rh-part weights, batch-duplicated rows
    wcrh_d = nc.dram_tensor("wcrh", [3, BH, D_H], F16, kind="ExternalInput").ap()
    bf_d = nc.dram_tensor("bf", [BH, 1], F32, kind="ExternalInput").ap()
    bu_d = nc.dram_tensor("bu", [BH, 1], F32, kind="ExternalInput").ap()
    bc_d = nc.dram_tensor("bcb", [BH, 1], F32, kind="ExternalInput").ap()
    out_d = nc.dram_tensor("out", [B_LOC, D_H, NN], F16,
                           kind="ExternalOutput").ap()

    aps = dict(at_d=at_d, zt_d=zt_d, xh_d=xh_d, hbs_d=hbs_d, wg0_d=wg0_d,
               wg1_d=wg1_d, wg2_d=wg2_d, wcx_d=wcx_d,
               wcrh_d=wcrh_d, bf_d=bf_d, bu_d=bu_d, bc_d=bc_d, out_d=out_d)
    with tile.TileContext(nc) as tc:
        _body(tc, aps)
    nc.compile()
    return nc


def _body(tc, aps):
    nc = tc.nc
    SIG = mybir.ActivationFunctionType.Sigmoid
    TANH = mybir.ActivationFunctionType.Tanh
    COPY = mybir.ActivationFunctionType.Copy
    at_d, zt_d, xh_d, hbs_d = (aps[k] for k in ("at_d", "zt_d", "xh_d", "hbs_d"))
    out_d = aps["out_d"]

    with (
        tc.tile_pool(name="const", bufs=1) as cpool,     # persistent tiles
        tc.tile_pool(name="band", bufs=2) as bpool,      # rotating band tiles
        tc.tile_pool(name="psum", bufs=8, space="PSUM") as pspool,
    ):
        # ---- persistent loads ----
        at8 = cpool.tile([CH, NG * NJ2 * 2 * BAND], F8, tag="at8")
        GW = NJ2 * 2 * BAND  # columns per group slab
        for g in range(NG):
            nc.sync.dma_start(out=at8[:, g * GW:(g + 1) * GW],
                              in_=at_d[:, g * GW:(g + 1) * GW])
        ztdr = cpool.tile([CH, NCHUNK * B_LOC * C], F8, tag="ztdr")
        nc.scalar.dma_start(out=ztdr[:], in_=zt_d[:])

        wg0_sb = [cpool.tile([C, D_H], F16, tag=f"wg0{k}", name=f"wg0{k}")
                  for k in range(2)]
        wg1_sb = [cpool.tile([C, D_H], F16, tag=f"wg1{k}", name=f"wg1{k}")
                  for k in range(2)]
        wg2_sb = [cpool.tile([C, D_H], F16, tag=f"wg2{k}", name=f"wg2{k}")
                  for k in range(2)]
        for k in range(2):
            nc.scalar.dma_start(out=wg0_sb[k][:], in_=aps["wg0_d"][k])
            nc.scalar.dma_start(out=wg1_sb[k][:], in_=aps["wg1_d"][k])
            nc.scalar.dma_start(out=wg2_sb[k][:], in_=aps["wg2_d"][k])
        wcx_sb = [cpool.tile([D_IN, D_H], F16, tag=f"wcx{k}", name=f"wcx{k}")
                  for k in range(3)]
        wcrh_sb = [cpool.tile([BH, D_H], F16, tag=f"wcrh{k}", name=f"wcrh{k}")
                   for k in range(3)]
        for k in range(3):
            nc.scalar.dma_start(out=wcx_sb[k][:], in_=aps["wcx_d"][k])
            nc.scalar.dma_start(out=wcrh_sb[k][:], in_=aps["wcrh_d"][k])
        bf_sb = cpool.tile([BH, 1], F32, tag="bf")
        nc.scalar.dma_start(out=bf_sb[:], in_=aps["bf_d"][:])
        bu_sb = cpool.tile([BH, 1], F32, tag="bu")
        nc.scalar.dma_start(out=bu_sb[:], in_=aps["bu_d"][:])
        bc_sb = cpool.tile([BH, 1], F32, tag="bc")
        nc.scalar.dma_start(out=bc_sb[:], in_=aps["bc_d"][:])

        # persistent activation tensors
        z1cm = [cpool.tile([C, NP], F16, tag=f"z1cm{b}", name=f"z1cm{b}")
                for b in range(B_LOC)]
        z1dr = cpool.tile([CH, NCHUNK * B_LOC * C], F8, tag="z1dr")
        rhdr = cpool.tile([CH, NCHUNK * BH], F8, tag="rhdr")
        zc1dr = cpool.tile([CH, NCHUNK * BH], F8, tag="zc1dr")
        u_st = cpool.tile([BH, NP], F16, tag="u_st")
        c_x = cpool.tile([BH, NP], F16, tag="c_x")

        def adj_mv(g, j2):
            base = (g * NJ2 + j2) * 2 * BAND
            return at8[:, base:base + 2 * BAND].rearrange(
                "p (i m) -> p i m", i=2)

        def zt_st(src, j2, b):
            base = j2 * 2 * B_LOC * C
            return src[:, base:base + 2 * B_LOC * C].rearrange(
                "p (i bc) -> p i bc", i=2)[:, :, b * C:(b + 1) * C]

        def bh_st(src, j2):
            base = j2 * 2 * BH
            return src[:, base:base + 2 * BH].rearrange(
                "p (i c) -> p i c", i=2)

        # ---- diffusion group helpers (all DoubleRow fp8, FD=512) ----
        def diff_gates(src, g, name):
            pss = []
            for b in range(B_LOC):
                ps = pspool.tile([C, BAND], F32, tag="ps", name=f"{name}{b}")
                for j2 in range(NJ2):
                    nc.tensor.matmul(ps[:, :], lhsT=zt_st(src, j2, b),
                                     rhs=adj_mv(g, j2), start=(j2 == 0),
                                     stop=(j2 == NJ2 - 1), perf_mode=DR)
                pss.append(ps)
            return pss

        def diff_cand(src, g, name):
            ps = pspool.tile([BH, BAND], F32, tag="ps", name=name)
            for j2 in range(NJ2):
                nc.tensor.matmul(ps[:, :], lhsT=bh_st(src, j2),
                                 rhs=adj_mv(g, j2), start=(j2 == 0),
                                 stop=(j2 == NJ2 - 1), perf_mode=DR)
            return ps

        def nm_store(band_sb, g, dst, width, bslice=None, scale=1.0):
            """DMA-transpose band_sb [P, 512] -> node-major fp16 temp, then
            DVE cast (x scale) into dst fp8 chunk-major columns for band g."""
            p = band_sb.shape[0]
            nm16 = bpool.tile([CH, 4 * p], F16, tag="nm16", name="nm16")
            nc.scalar.dma_start(
                out=nm16[:, :].rearrange("q (j c) -> q j c", j=4),
                in_=band_sb[:, :], transpose=True)
            base = g * 4 * width
            dst_ap = dst[:, base:base + 4 * width].rearrange(
                "q (j c) -> q j c", j=4)
            if bslice is not None:
                dst_ap = dst_ap[:, :, bslice]
            src_ap = nm16[:, :].rearrange("q (j c) -> q j c", j=4)
            nc.vector.tensor_scalar_mul(dst_ap, src_ap, scale)

        # ================= stage A: z1 = A z =================
        for g in range(NG):
            pss = diff_gates(ztdr, g, "psa")
            for b in range(B_LOC):
                nc.scalar.activation(z1cm[b][:, g * BAND:(g + 1) * BAND],
                                     pss[b][:, :], COPY, scale=CA)
                nm_store(z1cm[b][:, g * BAND:(g + 1) * BAND], g, z1dr,
                         B_LOC * C, bslice=slice(b * C, (b + 1) * C),
                         scale=S_Z1)

        # ================= stage B: z2 = A z1, fused gate convs ==========
        def conv_band(g):
            sl = slice(g * BAND, (g + 1) * BAND)
            xh_b = [bpool.tile([C, BAND], F16, tag=f"xhb{b}", name=f"xhb{b}")
                    for b in range(B_LOC)]
            for b in range(B_LOC):
                nc.sync.dma_start(out=xh_b[b][:, :], in_=xh_d[b][:, sl])
            h_b = bpool.tile([BH, BAND], F16, tag="h_b", name="h_b")
            nc.sync.dma_start(out=h_b[:, :], in_=hbs_d[:, sl])
            z2_b = z2bands[g % 2]
            psf = pspool.tile([BH, BAND], F32, tag="ps", name="psf")
            psu = pspool.tile([BH, BAND], F32, tag="ps", name="psu")
            pcx = pspool.tile([BH, BAND], F32, tag="ps", name="pcx")
            for b in range(B_LOC):
                rows = slice(b * D_H, (b + 1) * D_H)
                # order matters: z2-dependent matmuls last (z2 copy overlaps)
                nc.tensor.matmul(psf[rows, :], lhsT=wg0_sb[0],
                                 rhs=xh_b[b][:, :], start=True, stop=False)
                nc.tensor.matmul(psf[rows, :], lhsT=wg1_sb[0],
                                 rhs=z1cm[b][:, sl], start=False, stop=False)
                nc.tensor.matmul(psf[rows, :], lhsT=wg2_sb[0],
                                 rhs=z2_b[b][:, :], start=False, stop=True)
                nc.tensor.matmul(psu[rows, :], lhsT=wg0_sb[1],
                                 rhs=xh_b[b][:, :], start=True, stop=False)
                nc.tensor.matmul(psu[rows, :], lhsT=wg1_sb[1],
                                 rhs=z1cm[b][:, sl], start=False, stop=False)
                nc.tensor.matmul(psu[rows, :], lhsT=wg2_sb[1],
                                 rhs=z2_b[b][:, :], start=False, stop=True)
            rst = bpool.tile([BH, BAND], F16, tag="rst", name="rst")
            nc.scalar.activation(rst[:, :], psf[:, :], SIG, bias=bf_sb[:, :])
            nc.scalar.activation(u_st[:, sl], psu[:, :], SIG, bias=bu_sb[:, :])
            rh_b = bpool.tile([BH, BAND], F16, tag="rh_b", name="rh_b")
            nc.vector.tensor_mul(out=rh_b[:, :], in0=rst[:, :], in1=h_b[:, :])
            for b in range(B_LOC):
                rows = slice(b * D_H, (b + 1) * D_H)
                nc.tensor.matmul(pcx[rows, :], lhsT=wcx_sb[0],
                                 rhs=xh_b[b][0:D_IN, :], start=True, stop=False)
                nc.tensor.matmul(pcx[rows, :], lhsT=wcx_sb[1],
                                 rhs=z1cm[b][0:D_IN, sl], start=False, stop=False)
                nc.tensor.matmul(pcx[rows, :], lhsT=wcx_sb[2],
                                 rhs=z2_b[b][0:D_IN, :], start=False, stop=False)
                nc.tensor.matmul(pcx[rows, :], lhsT=wcrh_sb[0][rows, :],
                                 rhs=rh_b[rows, :], start=False, stop=True)
            nc.scalar.activation(c_x[:, sl], pcx[:, :], COPY)
            nm_store(rh_b, g, rhdr, BH, scale=S_RH)

        def diffB(g):
            pss = diff_gates(z1dr, g, "psb")
            z2_b = [bpool.tile([C, BAND], F16, tag=f"z2b{b}", name=f"z2b{b}")
                    for b in range(B_LOC)]
            for b in range(B_LOC):
                nc.scalar.activation(z2_b[b][:, :], pss[b][:, :], COPY,
                                     scale=CB)
            return z2_b

        # software pipeline: diffusion group g+1 issues before conv band g
        z2bands = [None, None]
        z2bands[0] = diffB(0)
        for g in range(NG):
            if g + 1 < NG:
                z2bands[(g + 1) % 2] = diffB(g + 1)
            conv_band(g)

        # ================= stage C: zc1 = A rh ==================
        def consC(g, ps):
            sl = slice(g * BAND, (g + 1) * BAND)
            zc1_b = bpool.tile([BH, BAND], F16, tag="zc1b", name="zc1b")
            nc.scalar.activation(zc1_b[:, :], ps[:, :], COPY, scale=CC)
            pc1 = pspool.tile([BH, BAND], F32, tag="ps", name="pc1")
            for b in range(B_LOC):
                rows = slice(b * D_H, (b + 1) * D_H)
                nc.tensor.matmul(pc1[rows, :], lhsT=wcrh_sb[1][rows, :],
                                 rhs=zc1_b[rows, :], start=True, stop=True)
            nc.vector.tensor_add(out=c_x[:, sl], in0=c_x[:, sl],
                                 in1=pc1[:, :])
            nm_store(zc1_b, g, zc1dr, BH, scale=S_ZC1)

        psC = diff_cand(rhdr, 0, "psc")
        for g in range(NG):
            psN = diff_cand(rhdr, g + 1, "psc") if g + 1 < NG else None
            consC(g, psC)
            psC = psN

        # ====== stage D: zc2 = A zc1, fused tanh + GRU combine ======
        def consD(g, ps):
            sl = slice(g * BAND, (g + 1) * BAND)
            zc2_b = bpool.tile([BH, BAND], F16, tag="zc2b", name="zc2b")
            nc.scalar.activation(zc2_b[:, :], ps[:, :], COPY, scale=CD)
            pc2 = pspool.tile([BH, BAND], F32, tag="ps", name="pc2")
            for b in range(B_LOC):
                rows = slice(b * D_H, (b + 1) * D_H)
                nc.tensor.matmul(pc2[rows, :], lhsT=wcrh_sb[2][rows, :],
                                 rhs=zc2_b[rows, :], start=True, stop=True)
            nc.vector.tensor_add(out=pc2[:, :], in0=pc2[:, :], in1=c_x[:, sl])
            cst = bpool.tile([BH, BAND], F16, tag="cst", name="cst")
            nc.scalar.activation(cst[:, :], pc2[:, :], TANH, bias=bc_sb[:, :])
            hD = bpool.tile([BH, BAND], F16, tag="h_b", name="hD")
            nc.sync.dma_start(out=hD[:, :], in_=hbs_d[:, sl])
            nc.vector.tensor_sub(out=hD[:, :], in0=hD[:, :], in1=cst[:, :])
            nc.vector.tensor_mul(out=hD[:, :], in0=u_st[:, sl], in1=hD[:, :])
            nc.vector.tensor_add(out=cst[:, :], in0=cst[:, :], in1=hD[:, :])
            w = min(BAND, NN - g * BAND)
            for b in range(B_LOC):
                nc.sync.dma_start(
                    out=out_d[b][:, g * BAND:g * BAND + w],
                    in_=cst[b * D_H:(b + 1) * D_H, 0:w])

        psD = diff_cand(zc1dr, 0, "psd")
        for g in range(NG):
            psN = diff_cand(zc1dr, g + 1, "psd") if g + 1 < NG else None
            consD(g, psD)
            psD = psN


# ---- host-side driver ----
_CACHED_NC = None
TRACE = False
TRACE_DIR = None
LAST_RESULTS = None


def _host_prep(x, h, adj, Wf, bf, Wu, bu, Wc, bc):
    A = np.zeros((NP, NP), dtype=np.float32)
    A[:NN, :NN] = adj * S_A
    at8 = np.ascontiguousarray(
        A.reshape(NG, BAND, NJ2, 2, CH).transpose(4, 0, 2, 3, 1)
        .reshape(CH, -1)).astype(NP8)

    WfT = Wf.T.astype(np.float16)     # [288, 64]
    WuT = Wu.T.astype(np.float16)
    WcT = Wc.T.astype(np.float16)
    wg0 = np.ascontiguousarray(np.stack([WfT[0:96], WuT[0:96]]))
    wg1 = np.ascontiguousarray(np.stack([WfT[96:192], WuT[96:192]]))
    wg2 = np.ascontiguousarray(np.stack([WfT[192:288], WuT[192:288]]))
    wcx = np.ascontiguousarray(np.stack([WcT[0:32], WcT[96:128],
                                         WcT[192:224]]))
    wcrh = np.ascontiguousarray(np.stack(
        [np.concatenate([WcT[32:96]] * B_LOC, axis=0),
         np.concatenate([WcT[128:192]] * B_LOC, axis=0),
         np.concatenate([WcT[224:288]] * B_LOC, axis=0)]))

    def bstack(v):
        return np.concatenate([v] * B_LOC).reshape(BH, 1).astype(np.float32)

    shared = {"at": at8, "wg0": wg0, "wg1": wg1, "wg2": wg2,
              "wcx": wcx, "wcrh": wcrh, "bf": bstack(bf), "bu": bstack(bu),
              "bcb": bstack(bc)}

    in_maps = []
    for core in range(NCORES):
        bs = slice(core * B_LOC, (core + 1) * B_LOC)
        zp = np.zeros((B_LOC, C, NP), dtype=np.float32)
        zp[:, :D_IN, :NN] = x[bs]
        zp[:, D_IN:, :NN] = h[bs]
        ztdr = np.ascontiguousarray(
            (zp.reshape(B_LOC, C, NCHUNK, CH).transpose(3, 2, 0, 1)
             .reshape(CH, -1)) * S_Z).astype(NP8)
        hbs = np.ascontiguousarray(
            zp[:, D_IN:, :].reshape(BH, NP)).astype(np.float16)
        in_maps.append(dict(shared, zt=ztdr, xh=zp.astype(np.float16),
                            hbs=hbs))
    return in_maps


def kernel(**inputs):
    global _CACHED_NC, LAST_RESULTS
    inputs = {k: np.asarray(v) for k, v in inputs.items()}
    if _CACHED_NC is None:
        _CACHED_NC = build_program()
    in_maps = _host_prep(**inputs)
    kw = {}
    if TRACE:
        kw = dict(trace=True, tmpdir=TRACE_DIR)
    res = run_bass_kernel_spmd(_CACHED_NC, in_maps,
                               core_ids=list(range(NCORES)), **kw)
    LAST_RESULTS = res
    outs = [res.results[i]["out"] for i in range(NCORES)]
    return np.concatenate(outs, axis=0).astype(np.float32)


if __name__ == "__main__":
    rng = np.random.default_rng(0)
    ins = {
        "x": rng.standard_normal((B, D_IN, NN), dtype=np.float32),
        "h": rng.standard_normal((B, D_H, NN), dtype=np.float32),
        "adj": rng.random((NN, NN), dtype=np.float32) / NN,
        "Wf": rng.standard_normal((D_H, 3 * C), dtype=np.float32) * 0.05,
        "Wu": rng.standard_normal((D_H, 3 * C), dtype=np.float32) * 0.05,
        "Wc": rng.standard_normal((D_H, 3 * C), dtype=np.float32) * 0.05,
        "bf": rng.standard_normal(D_H).astype(np.float32) * 0.05,
        "bu": rng.standard_normal(D_H).astype(np.float32) * 0.05,
        "bc": rng.standard_normal(D_H).astype(np.float32) * 0.05,
    }
    out = kernel(**ins)
    print(out.shape, out.dtype)


# revision 21
# speedup vs baseline: 1.7936x; 1.0745x over previous
"""GCGRU cell (order-2 graph diffusion GRU) Trainium2 Bass kernel, v2.

Strategy: data-parallel over batch (B=16 -> 2 batches per core x 8 cores).
The whole adjacency matrix lives RESIDENT in SBUF as scaled fp8 (16.8 MB =
128 KiB/partition), loaded once instead of being streamed 4x in fp16. All
four diffusion stages (z1 = A z, z2 = A z1, zc1 = A rh, zc2 = A zc1) run as
fp8 DoubleRow matmuls (K=256 per pass, FD=512) in sz-form: the activation
tensor is PE-stationary (node-major, fp8, DR-interleaved) and adj is the
wide moving operand. Outputs are born channel-major, feeding the 1x1 gate
convs (fp16) directly with no PE transposes. Node-major fp8 stationaries
for later stages are produced by DMA XBAR transposes + DVE casts, so the
transpose path costs no PE time. Candidate x-part conv terms are
pre-accumulated into c_x during the gate band loop; the final tanh/combine
is fused into stage D's group loop.

Scales (static): adj x2^14 (below fp8 subnormal range otherwise), z x8,
z1 x128 / rh x16 / zc1 x128 at their fp8 casts; each undone in the psum
descale of the following stage. Diffused features are ~100x smaller than
the direct z term in the convs, so fp8 error there is negligible.
"""

import numpy as np
import ml_dtypes

import concourse.bass as bass
from concourse import bacc
import concourse.mybir as mybir
import concourse.tile as tile
from concourse.bass_utils import run_bass_kernel_spmd

# problem constants
B, D_IN, D_H, NN = 16, 32, 64, 4000
NCORES = 8
B_LOC = B // NCORES          # batches per core
C = D_IN + D_H               # 96 channels into each gate conv
BH = B_LOC * D_H             # stacked batch-hidden rows (128)
NP = 4096                    # node dim padded to a multiple of 512
CH = 128                     # node chunk
NCHUNK = NP // CH            # 32 node chunks
NJ2 = NCHUNK // 2            # 16 DoubleRow chunk pairs
BAND = 512                   # psum group band (m-range per diffusion group)
NG = NP // BAND              # 8 groups
NP8 = ml_dtypes.float8_e4m3  # TRN fp8_e4m3

F8 = mybir.dt.float8e4
F16 = mybir.dt.float16
F32 = mybir.dt.float32
DR = mybir.MatmulPerfMode.DoubleRow

# static scales
S_A = float(2 ** 14)
S_Z = 8.0
S_Z1 = 128.0
S_RH = 16.0
S_ZC1 = 128.0
CA = 1.0 / (S_A * S_Z)       # stage A psum descale
CB = 1.0 / (S_A * S_Z1)      # stage B
CC = 1.0 / (S_A * S_RH)      # stage C
CD = 1.0 / (S_A * S_ZC1)     # stage D


def build_program():
    nc = bacc.Bacc("TRN2", target_bir_lowering=False, debug=False)

    # ---- DRAM I/O (all host-prepped layouts) ----
    # at8[p, ((g*NJ2+j2)*2+i)*BAND + m] = S_A * adj[g*BAND+m, j2*256+i*128+p]
    at_d = nc.dram_tensor("at", [CH, NG * NJ2 * 2 * BAND], F8,
                          kind="ExternalInput").ap()
    # ztdr[p, j*192 + b*96 + c] = S_Z * z[b, c, j*128+p],  z = [x;h]
    zt_d = nc.dram_tensor("zt", [CH, NCHUNK * B_LOC * C], F8,
                          kind="ExternalInput").ap()
    xh_d = nc.dram_tensor("xh", [B_LOC, C, NP], F16, kind="ExternalInput").ap()
    hbs_d = nc.dram_tensor("hbs", [BH, NP], F16, kind="ExternalInput").ap()
    # gate weights [Wf|Wu] fused per diffusion order; rows = contraction chans
    wg_d = nc.dram_tensor("wg", [3, C, 2 * D_H], F16, kind="ExternalInput").ap()
    # candidate x-part weights (orders 0,1,2; rows = x channels)
    wcx_d = nc.dram_tensor("wcx", [3, D_IN, D_H], F16, kind="ExternalInput").ap()
    # candidate rh-part weights, batch-duplicated rows
    wcrh_d = nc.dram_tensor("wcrh", [3, BH, D_H], F16, kind="ExternalInput").ap()
    bfu_d = nc.dram_tensor("bfu", [BH, 1], F32, kind="ExternalInput").ap()
    bc_d = nc.dram_tensor("bcb", [BH, 1], F32, kind="ExternalInput").ap()
    out_d = nc.dram_tensor("out", [B_LOC, D_H, NN], F16,
                           kind="ExternalOutput").ap()

    aps = dict(at_d=at_d, zt_d=zt_d, xh_d=xh_d, hbs_d=hbs_d, wg_d=wg_d,
               wcx_d=wcx_d, wcrh_d=wcrh_d, bfu_d=bfu_d, bc_d=bc_d,
               out_d=out_d)
    with tile.TileContext(nc) as tc:
        _body(tc, aps)
    nc.compile()
    return nc


def _body(tc, aps):
    nc = tc.nc
    SIG = mybir.ActivationFunctionType.Sigmoid
    TANH = mybir.ActivationFunctionType.Tanh
    COPY = mybir.ActivationFunctionType.Copy
    at_d, zt_d, xh_d, hbs_d = (aps[k] for k in ("at_d", "zt_d", "xh_d", "hbs_d"))
    out_d = aps["out_d"]

    with (
        tc.tile_pool(name="const", bufs=1) as cpool,     # persistent tiles
        tc.tile_pool(name="band", bufs=2) as bpool,      # rotating band tiles
        tc.tile_pool(name="psum", bufs=8, space="PSUM") as pspool,
    ):
        # ---- persistent loads ----
        # slab 0 quarter-granular (sync ring) + ztdr first on scalar so the
        # first diffusion matmuls can start ~3us in; later slabs split in
        # halves across both HWDGE rings for 2x trigger-path parallelism.
        at8 = cpool.tile([CH, NG * NJ2 * 2 * BAND], F8, tag="at8")
        GW = NJ2 * 2 * BAND  # columns per group slab
        ztdr = cpool.tile([CH, NCHUNK * B_LOC * C], F8, tag="ztdr")
        QW = GW // 4
        nc.sync.dma_start(out=at8[:, 0:QW], in_=at_d[:, 0:QW])
        nc.scalar.dma_start(out=ztdr[:], in_=zt_d[:])
        for q in range(1, 4):
            nc.sync.dma_start(out=at8[:, q * QW:(q + 1) * QW],
                              in_=at_d[:, q * QW:(q + 1) * QW])
        for g in range(1, NG):
            b0 = g * GW
            hw = GW // 2
            nc.sync.dma_start(out=at8[:, b0:b0 + hw], in_=at_d[:, b0:b0 + hw])
            nc.scalar.dma_start(out=at8[:, b0 + hw:b0 + GW],
                                in_=at_d[:, b0 + hw:b0 + GW])

        wg_sb = [cpool.tile([C, 2 * D_H], F16, tag=f"wg{k}", name=f"wg{k}")
                 for k in range(3)]
        wcx_sb = [cpool.tile([D_IN, D_H], F16, tag=f"wcx{k}", name=f"wcx{k}")
                  for k in range(3)]
        wcrh_sb = [cpool.tile([BH, D_H], F16, tag=f"wcrh{k}", name=f"wcrh{k}")
                   for k in range(3)]
        for k in range(3):
            nc.scalar.dma_start(out=wg_sb[k][:], in_=aps["wg_d"][k])
            nc.scalar.dma_start(out=wcx_sb[k][:], in_=aps["wcx_d"][k])
            nc.scalar.dma_start(out=wcrh_sb[k][:], in_=aps["wcrh_d"][k])
        bfu_sb = cpool.tile([BH, 1], F32, tag="bfu")
        nc.scalar.dma_start(out=bfu_sb[:], in_=aps["bfu_d"][:])
        bc_sb = cpool.tile([BH, 1], F32, tag="bc")
        nc.scalar.dma_start(out=bc_sb[:], in_=aps["bc_d"][:])

        # persistent activation tensors
        z1cm = [cpool.tile([C, NP], F16, tag=f"z1cm{b}", name=f"z1cm{b}")
                for b in range(B_LOC)]
        z1dr = cpool.tile([CH, NCHUNK * B_LOC * C], F8, tag="z1dr")
        rhdr = cpool.tile([CH, NCHUNK * BH], F8, tag="rhdr")
        zc1dr = cpool.tile([CH, NCHUNK * BH], F8, tag="zc1dr")
        u_st = cpool.tile([BH, NP], F16, tag="u_st")
        c_x = cpool.tile([BH, NP], F16, tag="c_x")

        def adj_mv(g, j2):
            base = (g * NJ2 + j2) * 2 * BAND
            return at8[:, base:base + 2 * BAND].rearrange(
                "p (i m) -> p i m", i=2)

        def zt_st(src, j2, b):
            base = j2 * 2 * B_LOC * C
            return src[:, base:base + 2 * B_LOC * C].rearrange(
                "p (i bc) -> p i bc", i=2)[:, :, b * C:(b + 1) * C]

        def bh_st(src, j2):
            base = j2 * 2 * BH
            return src[:, base:base + 2 * BH].rearrange(
                "p (i c) -> p i c", i=2)

        # ---- diffusion group helpers (all DoubleRow fp8, FD=512) ----
        def diff_gates(src, g, name):
            pss = []
            for b in range(B_LOC):
                ps = pspool.tile([C, BAND], F32, tag="ps", name=f"{name}{b}")
                for j2 in range(NJ2):
                    nc.tensor.matmul(ps[:, :], lhsT=zt_st(src, j2, b),
                                     rhs=adj_mv(g, j2), start=(j2 == 0),
                                     stop=(j2 == NJ2 - 1), perf_mode=DR)
                pss.append(ps)
            return pss

        def diff_cand(src, g, name):
            ps = pspool.tile([BH, BAND], F32, tag="ps", name=name)
            for j2 in range(NJ2):
                nc.tensor.matmul(ps[:, :], lhsT=bh_st(src, j2),
                                 rhs=adj_mv(g, j2), start=(j2 == 0),
                                 stop=(j2 == NJ2 - 1), perf_mode=DR)
            return ps

        def nm_store(band_sb, g, dst, width, bslice=None, scale=1.0):
            """DMA-transpose band_sb [P, 512] -> node-major fp16 temp, then
            DVE cast (x scale) into dst fp8 chunk-major columns for band g."""
            p = band_sb.shape[0]
            nm16 = bpool.tile([CH, 4 * p], F16, tag="nm16", name="nm16")
            nc.sync.dma_start(
                out=nm16[:, :].rearrange("q (j c) -> q j c", j=4),
                in_=band_sb[:, :], transpose=True)
            base = g * 4 * width
            dst_ap = dst[:, base:base + 4 * width].rearrange(
                "q (j c) -> q j c", j=4)
            if bslice is not None:
                dst_ap = dst_ap[:, :, bslice]
            src_ap = nm16[:, :].rearrange("q (j c) -> q j c", j=4)
            nc.vector.tensor_scalar_mul(dst_ap, src_ap, scale)

        # ================= stage A: z1 = A z =================
        for g in range(NG):
            pss = diff_gates(ztdr, g, "psa")
            for b in range(B_LOC):
                nc.vector.tensor_scalar_mul(
                    z1cm[b][:, g * BAND:(g + 1) * BAND], pss[b][:, :], CA)
                nm_store(z1cm[b][:, g * BAND:(g + 1) * BAND], g, z1dr,
                         B_LOC * C, bslice=slice(b * C, (b + 1) * C),
                         scale=S_Z1)

        # ================= stage B: z2 = A z1, fused gate convs ==========
        def conv_band(g):
            sl = slice(g * BAND, (g + 1) * BAND)
            xh_b = [bpool.tile([C, BAND], F16, tag=f"xhb{b}", name=f"xhb{b}")
                    for b in range(B_LOC)]
            h_b = [bpool.tile([D_H, BAND], F16, tag=f"hb{b}", name=f"hb{b}")
                   for b in range(B_LOC)]
            for b in range(B_LOC):
                nc.sync.dma_start(out=xh_b[b][:, :], in_=xh_d[b][:, sl])
                nc.sync.dma_start(out=h_b[b][:, :],
                                  in_=hbs_d[b * D_H:(b + 1) * D_H, sl])
            z2_b = z2bands[g % 2]
            # fused [Wf|Wu] stationaries: psum rows 0:64 = f, 64:128 = u
            psfu = [pspool.tile([BH, BAND], F32, tag="ps", name=f"psfu{b}")
                    for b in range(B_LOC)]
            pcx = pspool.tile([BH, BAND], F32, tag="ps", name="pcx")
            for b in range(B_LOC):
                # order matters: z2-dependent matmul last (z2 copy overlaps)
                nc.tensor.matmul(psfu[b][:, :], lhsT=wg_sb[0],
                                 rhs=xh_b[b][:, :], start=True, stop=False)
                nc.tensor.matmul(psfu[b][:, :], lhsT=wg_sb[1],
                                 rhs=z1cm[b][:, sl], start=False, stop=False)
                nc.tensor.matmul(psfu[b][:, :], lhsT=wg_sb[2],
                                 rhs=z2_b[b][:, :], start=False, stop=True)
            sig = [bpool.tile([BH, BAND], F16, tag=f"sig{b}", name=f"sig{b}")
                   for b in range(B_LOC)]
            rh_b = [bpool.tile([D_H, BAND], F16, tag=f"rhb{b}", name=f"rhb{b}")
                    for b in range(B_LOC)]
            for b in range(B_LOC):
                nc.scalar.activation(sig[b][:, :], psfu[b][:, :], SIG,
                                     bias=bfu_sb[:, :])
                nc.vector.tensor_mul(out=rh_b[b][:, :], in0=sig[b][0:D_H, :],
                                     in1=h_b[b][:, :])
            # u rows (64:128 of sig) into b-stacked u_st; b0 needs a
            # cross-quadrant move -> two 32-wide DVE copies
            nc.vector.tensor_copy(out=u_st[0:32, sl], in_=sig[0][64:96, :])
            nc.vector.tensor_copy(out=u_st[32:64, sl], in_=sig[0][96:128, :])
            nc.vector.tensor_copy(out=u_st[64:128, sl], in_=sig[1][64:128, :])
            for b in range(B_LOC):
                rows = slice(b * D_H, (b + 1) * D_H)
                nc.tensor.matmul(pcx[rows, :], lhsT=wcx_sb[0],
                                 rhs=xh_b[b][0:D_IN, :], start=True, stop=False)
                nc.tensor.matmul(pcx[rows, :], lhsT=wcx_sb[1],
                                 rhs=z1cm[b][0:D_IN, sl], start=False, stop=False)
                nc.tensor.matmul(pcx[rows, :], lhsT=wcx_sb[2],
                                 rhs=z2_b[b][0:D_IN, :], start=False, stop=False)
                nc.tensor.matmul(pcx[rows, :], lhsT=wcrh_sb[0][0:D_H, :],
                                 rhs=rh_b[b][:, :], start=False, stop=True)
            nc.scalar.activation(c_x[:, sl], pcx[:, :], COPY)
            for b in range(B_LOC):
                nm_store(rh_b[b], g, rhdr, BH,
                         bslice=slice(b * D_H, (b + 1) * D_H), scale=S_RH)

        def diffB(g):
            pss = diff_gates(z1dr, g, "psb")
            z2_b = [bpool.tile([C, BAND], F16, tag=f"z2b{b}", name=f"z2b{b}")
                    for b in range(B_LOC)]
            for b in range(B_LOC):
                nc.vector.tensor_scalar_mul(z2_b[b][:, :], pss[b][:, :], CB)
            return z2_b

        # software pipeline: diffusion group g+1 issues before conv band g
        z2bands = [None, None]
        z2bands[0] = diffB(0)
        for g in range(NG):
            if g + 1 < NG:
                z2bands[(g + 1) % 2] = diffB(g + 1)
            conv_band(g)

        # ================= stage C: zc1 = A rh ==================
        def consC(g, ps):
            sl = slice(g * BAND, (g + 1) * BAND)
            zc1_b = bpool.tile([BH, BAND], F16, tag="z2b1", name="zc1b")
            nc.scalar.activation(zc1_b[:, :], ps[:, :], COPY, scale=CC)
            pc1 = pspool.tile([BH, BAND], F32, tag="ps", name="pc1")
            for b in range(B_LOC):
                rows = slice(b * D_H, (b + 1) * D_H)
                nc.tensor.matmul(pc1[rows, :], lhsT=wcrh_sb[1][rows, :],
                                 rhs=zc1_b[rows, :], start=True, stop=True)
            nc.vector.tensor_add(out=c_x[:, sl], in0=c_x[:, sl],
                                 in1=pc1[:, :])
            nm_store(zc1_b, g, zc1dr, BH, scale=S_ZC1)

        psC = diff_cand(rhdr, 0, "psc")
        for g in range(NG):
            psN = diff_cand(rhdr, g + 1, "psc") if g + 1 < NG else None
            consC(g, psC)
            psC = psN

        # ====== stage D: zc2 = A zc1, fused tanh + GRU combine ======
        def consD(g, ps):
            sl = slice(g * BAND, (g + 1) * BAND)
            hD = bpool.tile([BH, BAND], F16, tag="xhb0", name="hD")
            nc.sync.dma_start(out=hD[:, :], in_=hbs_d[:, sl])
            zc2_b = bpool.tile([BH, BAND], F16, tag="xhb1", name="zc2b")
            nc.scalar.activation(zc2_b[:, :], ps[:, :], COPY, scale=CD)
            pc2 = pspool.tile([BH, BAND], F32, tag="ps", name="pc2")
            for b in range(B_LOC):
                rows = slice(b * D_H, (b + 1) * D_H)
                nc.tensor.matmul(pc2[rows, :], lhsT=wcrh_sb[2][rows, :],
                                 rhs=zc2_b[rows, :], start=True, stop=True)
            nc.vector.tensor_add(out=pc2[:, :], in0=pc2[:, :], in1=c_x[:, sl])
            cst = bpool.tile([BH, BAND], F16, tag="z2b0", name="cst")
            nc.scalar.activation(cst[:, :], pc2[:, :], TANH, bias=bc_sb[:, :])
            nc.vector.tensor_sub(out=hD[:, :], in0=hD[:, :], in1=cst[:, :])
            nc.vector.tensor_mul(out=hD[:, :], in0=u_st[:, sl], in1=hD[:, :])
            nc.vector.tensor_add(out=cst[:, :], in0=cst[:, :], in1=hD[:, :])
            w = min(BAND, NN - g * BAND)
            for b in range(B_LOC):
                nc.sync.dma_start(
                    out=out_d[b][:, g * BAND:g * BAND + w],
                    in_=cst[b * D_H:(b + 1) * D_H, 0:w])

        psD = diff_cand(zc1dr, 0, "psd")
        for g in range(NG):
            psN = diff_cand(zc1dr, g + 1, "psd") if g + 1 < NG else None
            consD(g, psD)
            psD = psN


# ---- host-side driver ----
_CACHED_NC = None
TRACE = False
TRACE_DIR = None
LAST_RESULTS = None


def _host_prep(x, h, adj, Wf, bf, Wu, bu, Wc, bc):
    A = np.zeros((NP, NP), dtype=np.float32)
    A[:NN, :NN] = adj * S_A
    at8 = np.ascontiguousarray(
        A.reshape(NG, BAND, NJ2, 2, CH).transpose(4, 0, 2, 3, 1)
        .reshape(CH, -1)).astype(NP8)

    WfT = Wf.T.astype(np.float16)     # [288, 64]
    WuT = Wu.T.astype(np.float16)
    WcT = Wc.T.astype(np.float16)
    wg = np.ascontiguousarray(np.stack(
        [np.concatenate([WfT[k * 96:(k + 1) * 96],
                         WuT[k * 96:(k + 1) * 96]], axis=1)
         for k in range(3)]))                              # [3, 96, 128]
    wcx = np.ascontiguousarray(np.stack([WcT[0:32], WcT[96:128],
                                         WcT[192:224]]))
    wcrh = np.ascontiguousarray(np.stack(
        [np.concatenate([WcT[32:96]] * B_LOC, axis=0),
         np.concatenate([WcT[128:192]] * B_LOC, axis=0),
         np.concatenate([WcT[224:288]] * B_LOC, axis=0)]))

    bfu = np.concatenate([bf, bu]).reshape(BH, 1).astype(np.float32)
    bc2 = np.concatenate([bc] * B_LOC).reshape(BH, 1).astype(np.float32)
    shared = {"at": at8, "wg": wg, "wcx": wcx, "wcrh": wcrh,
              "bfu": bfu, "bcb": bc2}

    in_maps = []
    for core in range(NCORES):
        bs = slice(core * B_LOC, (core + 1) * B_LOC)
        zp = np.zeros((B_LOC, C, NP), dtype=np.float32)
        zp[:, :D_IN, :NN] = x[bs]
        zp[:, D_IN:, :NN] = h[bs]
        ztdr = np.ascontiguousarray(
            (zp.reshape(B_LOC, C, NCHUNK, CH).transpose(3, 2, 0, 1)
             .reshape(CH, -1)) * S_Z).astype(NP8)
        hbs = np.ascontiguousarray(
            zp[:, D_IN:, :].reshape(BH, NP)).astype(np.float16)
        in_maps.append(dict(shared, zt=ztdr, xh=zp.astype(np.float16),
                            hbs=hbs))
    return in_maps


def kernel(**inputs):
    global _CACHED_NC, LAST_RESULTS
    inputs = {k: np.asarray(v) for k, v in inputs.items()}
    if _CACHED_NC is None:
        _CACHED_NC = build_program()
    in_maps = _host_prep(**inputs)
    kw = {}
    if TRACE:
        kw = dict(trace=True, tmpdir=TRACE_DIR)
    res = run_bass_kernel_spmd(_CACHED_NC, in_maps,
                               core_ids=list(range(NCORES)), **kw)
    LAST_RESULTS = res
    outs = [res.results[i]["out"] for i in range(NCORES)]
    return np.concatenate(outs, axis=0).astype(np.float32)


if __name__ == "__main__":
    rng = np.random.default_rng(0)
    ins = {
        "x": rng.standard_normal((B, D_IN, NN), dtype=np.float32),
        "h": rng.standard_normal((B, D_H, NN), dtype=np.float32),
        "adj": rng.random((NN, NN), dtype=np.float32) / NN,
        "Wf": rng.standard_normal((D_H, 3 * C), dtype=np.float32) * 0.05,
        "Wu": rng.standard_normal((D_H, 3 * C), dtype=np.float32) * 0.05,
        "Wc": rng.standard_normal((D_H, 3 * C), dtype=np.float32) * 0.05,
        "bf": rng.standard_normal(D_H).astype(np.float32) * 0.05,
        "bu": rng.standard_normal(D_H).astype(np.float32) * 0.05,
        "bc": rng.standard_normal(D_H).astype(np.float32) * 0.05,
    }
    out = kernel(**ins)
    print(out.shape, out.dtype)


# revision 22
# speedup vs baseline: 1.9122x; 1.0661x over previous
"""GCGRU cell (order-2 graph diffusion GRU) Trainium2 Bass kernel, v2.

Strategy: data-parallel over batch (B=16 -> 2 batches per core x 8 cores).
The whole adjacency matrix lives RESIDENT in SBUF as scaled fp8 (16.8 MB =
128 KiB/partition), loaded once instead of being streamed 4x in fp16. All
four diffusion stages (z1 = A z, z2 = A z1, zc1 = A rh, zc2 = A zc1) run as
fp8 DoubleRow matmuls (K=256 per pass, FD=512) in sz-form: the activation
tensor is PE-stationary (node-major, fp8, DR-interleaved) and adj is the
wide moving operand. Outputs are born channel-major, feeding the 1x1 gate
convs (fp16) directly with no PE transposes. Node-major fp8 stationaries
for later stages are produced by DMA XBAR transposes + DVE casts, so the
transpose path costs no PE time. Gate convs use fused [Wf|Wu] stationaries;
candidate x-part conv terms are pre-accumulated into c_x during the gate
band loop; the final tanh/combine is fused into stage D's group loop.

DMA plan: at8 slabs split j2-granular at the head (PE starts ~8us in) and
6/10 across the sync/scalar HWDGE rings (sync also carries the band
transposes). Activation band tiles (xh, h) prefetch one band ahead on the
scalar ring.

Scales (static): adj x2^14 (below fp8 subnormal range otherwise), z x8,
z1 x128 / rh x16 / zc1 x128 at their fp8 casts; each undone in the psum
descale of the following stage. Diffused features are ~100x smaller than
the direct z term in the convs, so fp8 error there is negligible.
"""

import numpy as np
import ml_dtypes

import concourse.bass as bass
from concourse import bacc
import concourse.mybir as mybir
import concourse.tile as tile
from concourse.bass_utils import run_bass_kernel_spmd

# problem constants
B, D_IN, D_H, NN = 16, 32, 64, 4000
NCORES = 8
B_LOC = B // NCORES          # batches per core
C = D_IN + D_H               # 96 channels into each gate conv
BH = B_LOC * D_H             # stacked batch-hidden rows (128)
NP = 4096                    # node dim padded to a multiple of 512
CH = 128                     # node chunk
NCHUNK = NP // CH            # 32 node chunks
NJ2 = NCHUNK // 2            # 16 DoubleRow chunk pairs
BAND = 512                   # psum group band (m-range per diffusion group)
NG = NP // BAND              # 8 groups
NP8 = ml_dtypes.float8_e4m3  # TRN fp8_e4m3

F8 = mybir.dt.float8e4
F16 = mybir.dt.float16
F32 = mybir.dt.float32
DR = mybir.MatmulPerfMode.DoubleRow

# static scales
S_A = float(2 ** 14)
S_Z = 8.0
S_Z1 = 128.0
S_RH = 16.0
S_ZC1 = 128.0
CA = 1.0 / (S_A * S_Z)       # stage A psum descale
CB = 1.0 / (S_A * S_Z1)      # stage B
CC = 1.0 / (S_A * S_RH)      # stage C
CD = 1.0 / (S_A * S_ZC1)     # stage D


def build_program():
    nc = bacc.Bacc("TRN2", target_bir_lowering=False, debug=False)

    # ---- DRAM I/O (all host-prepped layouts) ----
    # at8[p, ((g*NJ2+j2)*2+i)*BAND + m] = S_A * adj[g*BAND+m, j2*256+i*128+p]
    at_d = nc.dram_tensor("at", [CH, NG * NJ2 * 2 * BAND], F8,
                          kind="ExternalInput").ap()
    # ztdr[p, j*192 + b*96 + c] = S_Z * z[b, c, j*128+p],  z = [x;h]
    zt_d = nc.dram_tensor("zt", [CH, NCHUNK * B_LOC * C], F8,
                          kind="ExternalInput").ap()
    xh_d = nc.dram_tensor("xh", [B_LOC, C, NP], F16, kind="ExternalInput").ap()
    hbs_d = nc.dram_tensor("hbs", [BH, NP], F16, kind="ExternalInput").ap()
    # gate weights [Wf|Wu] fused per diffusion order; rows = contraction chans
    wg_d = nc.dram_tensor("wg", [3, C, 2 * D_H], F16, kind="ExternalInput").ap()
    # candidate x-part weights (orders 0,1,2; rows = x channels)
    wcx_d = nc.dram_tensor("wcx", [3, D_IN, D_H], F16, kind="ExternalInput").ap()
    # candidate rh-part weights, batch-duplicated rows
    wcrh_d = nc.dram_tensor("wcrh", [3, BH, D_H], F16, kind="ExternalInput").ap()
    bfu_d = nc.dram_tensor("bfu", [BH, 1], F32, kind="ExternalInput").ap()
    bc_d = nc.dram_tensor("bcb", [BH, 1], F32, kind="ExternalInput").ap()
    out_d = nc.dram_tensor("out", [B_LOC, D_H, NN], F16,
                           kind="ExternalOutput").ap()

    aps = dict(at_d=at_d, zt_d=zt_d, xh_d=xh_d, hbs_d=hbs_d, wg_d=wg_d,
               wcx_d=wcx_d, wcrh_d=wcrh_d, bfu_d=bfu_d, bc_d=bc_d,
               out_d=out_d)
    with tile.TileContext(nc) as tc:
        _body(tc, aps)
    nc.compile()
    return nc


def _body(tc, aps):
    nc = tc.nc
    SIG = mybir.ActivationFunctionType.Sigmoid
    TANH = mybir.ActivationFunctionType.Tanh
    COPY = mybir.ActivationFunctionType.Copy
    at_d, zt_d, xh_d, hbs_d = (aps[k] for k in ("at_d", "zt_d", "xh_d", "hbs_d"))
    out_d = aps["out_d"]

    with (
        tc.tile_pool(name="const", bufs=1) as cpool,     # persistent tiles
        tc.tile_pool(name="band", bufs=2) as bpool,      # rotating band tiles
        tc.tile_pool(name="psum", bufs=8, space="PSUM") as pspool,
    ):
        # ---- persistent loads ----
        # j2-granular head so the first diffusion matmuls start ~8us in;
        # then 6/10 j2-unit split of each slab across the sync/scalar rings
        # (sync also carries the band transposes).
        at8 = cpool.tile([CH, NG * NJ2 * 2 * BAND], F8, tag="at8")
        GW = NJ2 * 2 * BAND  # columns per group slab
        JW = 2 * BAND        # columns per j2 unit
        ztdr = cpool.tile([CH, NCHUNK * B_LOC * C], F8, tag="ztdr")
        ZQ = NCHUNK * B_LOC * C // 4
        # head: slab 0 paced against ztdr quarters
        for q in range(3):
            nc.sync.dma_start(out=at8[:, q * 2 * JW:(q + 1) * 2 * JW],
                              in_=at_d[:, q * 2 * JW:(q + 1) * 2 * JW])
        for q in range(4):
            nc.scalar.dma_start(out=ztdr[:, q * ZQ:(q + 1) * ZQ],
                                in_=zt_d[:, q * ZQ:(q + 1) * ZQ])
        nc.scalar.dma_start(out=at8[:, 6 * JW:GW], in_=at_d[:, 6 * JW:GW])
        for g in range(1, NG):
            b0 = g * GW
            hw = 6 * JW
            nc.sync.dma_start(out=at8[:, b0:b0 + hw], in_=at_d[:, b0:b0 + hw])
            nc.scalar.dma_start(out=at8[:, b0 + hw:b0 + GW],
                                in_=at_d[:, b0 + hw:b0 + GW])

        wg_sb = [cpool.tile([C, 2 * D_H], F16, tag=f"wg{k}", name=f"wg{k}")
                 for k in range(3)]
        wcx_sb = [cpool.tile([D_IN, D_H], F16, tag=f"wcx{k}", name=f"wcx{k}")
                  for k in range(3)]
        wcrh_sb = [cpool.tile([BH, D_H], F16, tag=f"wcrh{k}", name=f"wcrh{k}")
                   for k in range(3)]
        for k in range(3):
            nc.scalar.dma_start(out=wg_sb[k][:], in_=aps["wg_d"][k])
            nc.scalar.dma_start(out=wcx_sb[k][:], in_=aps["wcx_d"][k])
            nc.scalar.dma_start(out=wcrh_sb[k][:], in_=aps["wcrh_d"][k])
        bfu_sb = cpool.tile([BH, 1], F32, tag="bfu")
        nc.scalar.dma_start(out=bfu_sb[:], in_=aps["bfu_d"][:])
        bc_sb = cpool.tile([BH, 1], F32, tag="bc")
        nc.scalar.dma_start(out=bc_sb[:], in_=aps["bc_d"][:])

        # persistent activation tensors; z1cm columns are (g, b, m)-ordered
        # so one DMA transpose per band covers both batches
        z1cm = cpool.tile([C, B_LOC * NP], F16, tag="z1cm")
        z1dr = cpool.tile([CH, NCHUNK * B_LOC * C], F8, tag="z1dr")
        rhdr = cpool.tile([CH, NCHUNK * BH], F8, tag="rhdr")
        zc1dr = cpool.tile([CH, NCHUNK * BH], F8, tag="zc1dr")
        u_st = cpool.tile([BH, NP], F16, tag="u_st")
        c_x = cpool.tile([BH, NP], F16, tag="c_x")

        def z1sl(g, b):
            return z1cm[:, (2 * g + b) * BAND:(2 * g + b + 1) * BAND]

        def adj_mv(g, j2):
            base = (g * NJ2 + j2) * 2 * BAND
            return at8[:, base:base + 2 * BAND].rearrange(
                "p (i m) -> p i m", i=2)

        def zt_st(src, j2, b):
            base = j2 * 2 * B_LOC * C
            return src[:, base:base + 2 * B_LOC * C].rearrange(
                "p (i bc) -> p i bc", i=2)[:, :, b * C:(b + 1) * C]

        def bh_st(src, j2):
            base = j2 * 2 * BH
            return src[:, base:base + 2 * BH].rearrange(
                "p (i c) -> p i c", i=2)

        # ---- diffusion group helpers (all DoubleRow fp8, FD=512) ----
        def diff_gates(src, g, name):
            pss = []
            for b in range(B_LOC):
                ps = pspool.tile([C, BAND], F32, tag="ps", name=f"{name}{b}")
                for j2 in range(NJ2):
                    nc.tensor.matmul(ps[:, :], lhsT=zt_st(src, j2, b),
                                     rhs=adj_mv(g, j2), start=(j2 == 0),
                                     stop=(j2 == NJ2 - 1), perf_mode=DR)
                pss.append(ps)
            return pss

        def diff_cand(src, g, name):
            ps = pspool.tile([BH, BAND], F32, tag="ps", name=name)
            for j2 in range(NJ2):
                nc.tensor.matmul(ps[:, :], lhsT=bh_st(src, j2),
                                 rhs=adj_mv(g, j2), start=(j2 == 0),
                                 stop=(j2 == NJ2 - 1), perf_mode=DR)
            return ps

        def nm_store(band_sb, g, dst, scale):
            """DMA-transpose band_sb [128, 512] (bh-stacked) -> fp16 temp,
            then DVE cast (x scale) into dst fp8 band-g columns."""
            nm16 = bpool.tile([CH, 4 * BH], F16, tag="nm16", name="nm16")
            nc.sync.dma_start(
                out=nm16[:, :].rearrange("q (j c) -> q j c", j=4),
                in_=band_sb[:, :], transpose=True)
            base = g * 4 * BH
            nc.vector.tensor_scalar_mul(
                dst[:, base:base + 4 * BH].rearrange("q (j c) -> q j c", j=4),
                nm16[:, :].rearrange("q (j c) -> q j c", j=4), scale)

        def nm_store2(band_sb, g, dst, scale):
            """Combined two-batch DMA-transpose: band_sb [P, 2*BAND] with
            (b, m) columns -> per-batch DVE casts into dst fp8 band-g cols
            (dst column layout j*(2P) + b*P + c)."""
            p = band_sb.shape[0]
            nm16 = bpool.tile([CH, 8 * p], F16, tag="nm16", name="nm16")
            nc.sync.dma_start(
                out=nm16[:, :].rearrange("q (jj c) -> q jj c", jj=8),
                in_=band_sb[:, :], transpose=True)
            base = g * 4 * 2 * p
            for b in range(B_LOC):
                dst_ap = dst[:, base:base + 8 * p].rearrange(
                    "q (j bc) -> q j bc", j=4)[:, :, b * p:(b + 1) * p]
                src_ap = nm16[:, b * 4 * p:(b + 1) * 4 * p].rearrange(
                    "q (j c) -> q j c", j=4)
                nc.vector.tensor_scalar_mul(dst_ap, src_ap, scale)

        # ================= stage A: z1 = A z =================
        for g in range(NG):
            pss = diff_gates(ztdr, g, "psa")
            for b in range(B_LOC):
                nc.vector.tensor_scalar_mul(z1sl(g, b), pss[b][:, :], CA)
            nm_store2(z1cm[:, g * 2 * BAND:(g + 1) * 2 * BAND], g, z1dr,
                      S_Z1)

        # ================= stage B: z2 = A z1, fused gate convs ==========
        def loads(g):
            sl = slice(g * BAND, (g + 1) * BAND)
            xh_b = [bpool.tile([C, BAND], F16, tag=f"xhb{b}", name=f"xhb{b}")
                    for b in range(B_LOC)]
            h_b = [bpool.tile([D_H, BAND], F16, tag=f"hb{b}", name=f"hb{b}")
                   for b in range(B_LOC)]
            for b in range(B_LOC):
                nc.scalar.dma_start(out=xh_b[b][:, :], in_=xh_d[b][:, sl])
                nc.scalar.dma_start(out=h_b[b][:, :],
                                    in_=hbs_d[b * D_H:(b + 1) * D_H, sl])
            return xh_b, h_b

        def conv_band(g, xh_b, h_b):
            sl = slice(g * BAND, (g + 1) * BAND)
            z2_b = z2bands[g % 2]
            # fused [Wf|Wu] stationaries: psum rows 0:64 = f, 64:128 = u
            psfu = [pspool.tile([BH, BAND], F32, tag="ps", name=f"psfu{b}")
                    for b in range(B_LOC)]
            pcx = pspool.tile([BH, BAND], F32, tag="ps", name="pcx")
            for b in range(B_LOC):
                # order matters: z2-dependent matmul last (z2 copy overlaps)
                nc.tensor.matmul(psfu[b][:, :], lhsT=wg_sb[0],
                                 rhs=xh_b[b][:, :], start=True, stop=False)
                nc.tensor.matmul(psfu[b][:, :], lhsT=wg_sb[1],
                                 rhs=z1sl(g, b), start=False, stop=False)
                nc.tensor.matmul(psfu[b][:, :], lhsT=wg_sb[2],
                                 rhs=z2_b[b][:, :], start=False, stop=True)
            sig = [bpool.tile([BH, BAND], F16, tag=f"sig{b}", name=f"sig{b}")
                   for b in range(B_LOC)]
            rh = bpool.tile([D_H, 2 * BAND], F16, tag="rh", name="rh")
            for b in range(B_LOC):
                nc.scalar.activation(sig[b][:, :], psfu[b][:, :], SIG,
                                     bias=bfu_sb[:, :])
                nc.vector.tensor_mul(
                    out=rh[:, b * BAND:(b + 1) * BAND],
                    in0=sig[b][0:D_H, :], in1=h_b[b][:, :])
            # u rows (64:128 of sig) into b-stacked u_st; b0 needs a
            # cross-quadrant move -> two 32-wide DVE copies
            nc.vector.tensor_copy(out=u_st[0:32, sl], in_=sig[0][64:96, :])
            nc.vector.tensor_copy(out=u_st[32:64, sl], in_=sig[0][96:128, :])
            nc.vector.tensor_copy(out=u_st[64:128, sl], in_=sig[1][64:128, :])
            for b in range(B_LOC):
                rows = slice(b * D_H, (b + 1) * D_H)
                nc.tensor.matmul(pcx[rows, :], lhsT=wcx_sb[0],
                                 rhs=xh_b[b][0:D_IN, :], start=True, stop=False)
                nc.tensor.matmul(pcx[rows, :], lhsT=wcx_sb[1],
                                 rhs=z1sl(g, b)[0:D_IN, :], start=False,
                                 stop=False)
                nc.tensor.matmul(pcx[rows, :], lhsT=wcx_sb[2],
                                 rhs=z2_b[b][0:D_IN, :], start=False, stop=False)
                nc.tensor.matmul(pcx[rows, :], lhsT=wcrh_sb[0][0:D_H, :],
                                 rhs=rh[:, b * BAND:(b + 1) * BAND],
                                 start=False, stop=True)
            nc.scalar.activation(c_x[:, sl], pcx[:, :], COPY)
            nm_store2(rh, g, rhdr, S_RH)

        def diffB(g):
            pss = diff_gates(z1dr, g, "psb")
            z2_b = [bpool.tile([C, BAND], F16, tag=f"z2b{b}", name=f"z2b{b}")
                    for b in range(B_LOC)]
            for b in range(B_LOC):
                nc.scalar.activation(z2_b[b][:, :], pss[b][:, :], COPY,
                                     scale=CB)
            return z2_b

        # software pipeline: loads + diffusion group g+1 issue before the
        # band-g convs
        z2bands = [None, None]
        ld = loads(0)
        z2bands[0] = diffB(0)
        for g in range(NG):
            ldn = None
            if g + 1 < NG:
                ldn = loads(g + 1)
                z2bands[(g + 1) % 2] = diffB(g + 1)
            conv_band(g, *ld)
            ld = ldn

        # ================= stage C: zc1 = A rh ==================
        def consC(g, ps):
            sl = slice(g * BAND, (g + 1) * BAND)
            zc1_b = bpool.tile([BH, BAND], F16, tag="z2b1", name="zc1b")
            nc.scalar.activation(zc1_b[:, :], ps[:, :], COPY, scale=CC)
            pc1 = pspool.tile([BH, BAND], F32, tag="ps", name="pc1")
            for b in range(B_LOC):
                rows = slice(b * D_H, (b + 1) * D_H)
                nc.tensor.matmul(pc1[rows, :], lhsT=wcrh_sb[1][rows, :],
                                 rhs=zc1_b[rows, :], start=True, stop=True)
            nc.vector.tensor_add(out=c_x[:, sl], in0=c_x[:, sl],
                                 in1=pc1[:, :])
            nm_store(zc1_b, g, zc1dr, S_ZC1)

        psC = diff_cand(rhdr, 0, "psc")
        for g in range(NG):
            psN = diff_cand(rhdr, g + 1, "psc") if g + 1 < NG else None
            consC(g, psC)
            psC = psN

        # ====== stage D: zc2 = A zc1, fused tanh + GRU combine ======
        def consD(g, ps):
            sl = slice(g * BAND, (g + 1) * BAND)
            hD = bpool.tile([BH, BAND], F16, tag="xhb0", name="hD")
            nc.sync.dma_start(out=hD[:, :], in_=hbs_d[:, sl])
            zc2_b = bpool.tile([BH, BAND], F16, tag="xhb1", name="zc2b")
            nc.scalar.activation(zc2_b[:, :], ps[:, :], COPY, scale=CD)
            pc2 = pspool.tile([BH, BAND], F32, tag="ps", name="pc2")
            for b in range(B_LOC):
                rows = slice(b * D_H, (b + 1) * D_H)
                nc.tensor.matmul(pc2[rows, :], lhsT=wcrh_sb[2][rows, :],
                                 rhs=zc2_b[rows, :], start=True, stop=True)
            nc.vector.tensor_add(out=pc2[:, :], in0=pc2[:, :], in1=c_x[:, sl])
            cst = bpool.tile([BH, BAND], F16, tag="z2b0", name="cst")
            nc.scalar.activation(cst[:, :], pc2[:, :], TANH, bias=bc_sb[:, :])
            nc.vector.tensor_sub(out=hD[:, :], in0=hD[:, :], in1=cst[:, :])
            nc.vector.tensor_mul(out=hD[:, :], in0=u_st[:, sl], in1=hD[:, :])
            nc.vector.tensor_add(out=cst[:, :], in0=cst[:, :], in1=hD[:, :])
            w = min(BAND, NN - g * BAND)
            for b in range(B_LOC):
                nc.sync.dma_start(
                    out=out_d[b][:, g * BAND:g * BAND + w],
                    in_=cst[b * D_H:(b + 1) * D_H, 0:w])

        psD = diff_cand(zc1dr, 0, "psd")
        for g in range(NG):
            psN = diff_cand(zc1dr, g + 1, "psd") if g + 1 < NG else None
            consD(g, psD)
            psD = psN


# ---- host-side driver ----
_CACHED_NC = None
TRACE = False
TRACE_DIR = None
LAST_RESULTS = None


def _host_prep(x, h, adj, Wf, bf, Wu, bu, Wc, bc):
    A = np.zeros((NP, NP), dtype=np.float32)
    A[:NN, :NN] = adj * S_A
    at8 = np.ascontiguousarray(
        A.reshape(NG, BAND, NJ2, 2, CH).transpose(4, 0, 2, 3, 1)
        .reshape(CH, -1)).astype(NP8)

    WfT = Wf.T.astype(np.float16)     # [288, 64]
    WuT = Wu.T.astype(np.float16)
    WcT = Wc.T.astype(np.float16)
    wg = np.ascontiguousarray(np.stack(
        [np.concatenate([WfT[k * 96:(k + 1) * 96],
                         WuT[k * 96:(k + 1) * 96]], axis=1)
         for k in range(3)]))                              # [3, 96, 128]
    wcx = np.ascontiguousarray(np.stack([WcT[0:32], WcT[96:128],
                                         WcT[192:224]]))
    wcrh = np.ascontiguousarray(np.stack(
        [np.concatenate([WcT[32:96]] * B_LOC, axis=0),
         np.concatenate([WcT[128:192]] * B_LOC, axis=0),
         np.concatenate([WcT[224:288]] * B_LOC, axis=0)]))

    bfu = np.concatenate([bf, bu]).reshape(BH, 1).astype(np.float32)
    bc2 = np.concatenate([bc] * B_LOC).reshape(BH, 1).astype(np.float32)
    shared = {"at": at8, "wg": wg, "wcx": wcx, "wcrh": wcrh,
              "bfu": bfu, "bcb": bc2}

    in_maps = []
    for core in range(NCORES):
        bs = slice(core * B_LOC, (core + 1) * B_LOC)
        zp = np.zeros((B_LOC, C, NP), dtype=np.float32)
        zp[:, :D_IN, :NN] = x[bs]
        zp[:, D_IN:, :NN] = h[bs]
        ztdr = np.ascontiguousarray(
            (zp.reshape(B_LOC, C, NCHUNK, CH).transpose(3, 2, 0, 1)
             .reshape(CH, -1)) * S_Z).astype(NP8)
        hbs = np.ascontiguousarray(
            zp[:, D_IN:, :].reshape(BH, NP)).astype(np.float16)
        in_maps.append(dict(shared, zt=ztdr, xh=zp.astype(np.float16),
                            hbs=hbs))
    return in_maps


def kernel(**inputs):
    global _CACHED_NC, LAST_RESULTS
    inputs = {k: np.asarray(v) for k, v in inputs.items()}
    if _CACHED_NC is None:
        _CACHED_NC = build_program()
    in_maps = _host_prep(**inputs)
    kw = {}
    if TRACE:
        kw = dict(trace=True, tmpdir=TRACE_DIR)
    res = run_bass_kernel_spmd(_CACHED_NC, in_maps,
                               core_ids=list(range(NCORES)), **kw)
    LAST_RESULTS = res
    outs = [res.results[i]["out"] for i in range(NCORES)]
    return np.concatenate(outs, axis=0).astype(np.float32)


if __name__ == "__main__":
    rng = np.random.default_rng(0)
    ins = {
        "x": rng.standard_normal((B, D_IN, NN), dtype=np.float32),
        "h": rng.standard_normal((B, D_H, NN), dtype=np.float32),
        "adj": rng.random((NN, NN), dtype=np.float32) / NN,
        "Wf": rng.standard_normal((D_H, 3 * C), dtype=np.float32) * 0.05,
        "Wu": rng.standard_normal((D_H, 3 * C), dtype=np.float32) * 0.05,
        "Wc": rng.standard_normal((D_H, 3 * C), dtype=np.float32) * 0.05,
        "bf": rng.standard_normal(D_H).astype(np.float32) * 0.05,
        "bu": rng.standard_normal(D_H).astype(np.float32) * 0.05,
        "bc": rng.standard_normal(D_H).astype(np.float32) * 0.05,
    }
    out = kernel(**ins)
    print(out.shape, out.dtype)
